# revision 47
# baseline (speedup 1.0000x reference)
# Multi-head attention (N=4, S=2048, E=512, H=8, D=64) on 8 NeuronCores.
#
# Sharding: core c -> (batch n = c//2, query half qh = c%2). Each core
# computes attention for its 1024 query rows against the valid keys of
# its batch, all 8 heads, and the full output projection for its rows, so
# outputs are disjoint and no collectives are needed.
#
# Host-side preprocessing (exact, done once per call):
#   - Key compaction: the mask is per-batch over keys; masked keys get
#     exp(-1e9/8) == 0.0f in the reference, so they contribute nothing.
#     K/V/mask are gathered to the valid keys and zero-padded to SK_PAD
#     rows (P(overflow) ~ 5.7 sigma per batch; falls back to the full
#     2048-key module if it ever happens).
#   - Weight folding (pure weight algebra):
#       A  = Wq^T @ Wk / 8          scores = (Xq @ A) @ Xk^T
#       M_h = Wv^T @ Wo[:, h]^T     out += (attn @ Xv_h) @ M_h
#       btot = bo + Wo @ tile(bv,8) exact because attention rows sum to 1
#       bk-term cancels in softmax; bq-term via per-k bias (zero here).
#     A is stored block-diagonally [128,128] so one matmul projects a
#     head pair at once.
#
# Device per core:
#   - PE-transpose Q,K chunks (bf16) to [e, s] layout
#   - XqA^T = blockdiag(A)^T-proj of Q^T per head pair
#   - scores^T[k,q] = K_h^T.T @ XqA_h^T  (PSUM f32)
#   - exp on ACT straight from PSUM with per-partition mask bias -> bf16
#   - AV^T = V_aug.T @ exp^T with a ones column in V giving softmax
#     denominators as row 64; reciprocal on ACT (same act table as Exp),
#     broadcast across partitions with a rank-1 f32r matmul, multiply.
#   - out[q,:] = sum_h AVT_h.T @ M_h (+btot): head pairs at partition
#     bases 0/64 make this a single 128-contraction matmul per slot.

import numpy as np
import ml_dtypes

import concourse.bass as bass
import concourse.tile as tile
from concourse import bacc, mybir
from concourse.bass_utils import run_bass_kernel_spmd
from concourse.masks import make_identity

F32 = mybir.dt.float32
BF16 = mybir.dt.bfloat16
I32 = mybir.dt.int32

H = 8
D = 64
E = 512
N_CORES = 8
FULL_N, FULL_S = 4, 2048
SQ = 1024            # per-core query rows
SK_PAD = 1152        # compacted+padded key rows (9 chunks of 128)
MASK_BIAS = -1.25e8  # == -1e9 / sqrt(64), applied pre-softmax

# Engine facts probed on this runtime: Pool partition_broadcast and custom-DVE
# ops produce garbage / fail ISA checks; DVE divide and Pool
# scalar_tensor_tensor are rejected by walrus; DVE fp8 converts and fp8
# DoubleRow matmuls DO work; stream_shuffle semantics: out[j+32k]=in[mask[j]+32k].


def _split_even(n_items, n_steps):
    """Distribute range(n_items) into n_steps contiguous chunks, sizes
    as even as possible (larger first)."""
    out, base = [], 0
    for s in range(n_steps):
        take = (n_items - base + (n_steps - s - 1)) // (n_steps - s)
        out.append(list(range(base, base + take)))
        base += take
    return out


def _emit(tc, t, SQ, SK, has_qbias):
    nc = tc.nc
    NQC = SQ // 128           # query chunks (transpose granularity)
    NKC = SK // 128           # key chunks
    QGS = min(512, SQ)        # q group size for matmul free dim
    NQG = SQ // QGS
    H0 = (NKC + 1) // 2       # k-chunks in half 0 (exp half-tile)
    H1 = NKC - H0
    F32R = mybir.dt.float32r
    sub, mult, add = (mybir.AluOpType.subtract, mybir.AluOpType.mult,
                      mybir.AluOpType.add)

    with (
        tc.tile_pool(name="singles", bufs=1) as singles,
        tc.tile_pool(name="stage", bufs=6) as stage,
        tc.tile_pool(name="expp", bufs=7) as expp,
        tc.tile_pool(name="small", bufs=2) as small,
        tc.tile_pool(name="outp", bufs=2) as outp,
        tc.tile_pool(name="p_sc", bufs=2, space="PSUM") as p_sc,
        tc.tile_pool(name="p_misc", bufs=4, space="PSUM") as p_misc,
    ):
        # ---- constants / weights ----
        ident = singles.tile([128, 128], BF16)
        make_identity(nc, ident)
        identf = singles.tile([128, 128], F32)
        nc.vector.tensor_copy(identf, ident)

        # ~3us of dummy matmuls: the PE p-state ramps to max (2.4GHz) after
        # 3us of continuous execution, so the real transposes/scores start
        # at full clock instead of 0.65GHz
        warm = p_misc.tile([128, 128], F32, tag="ps")
        for _ in range(24):
            nc.tensor.matmul(warm, lhsT=ident, rhs=ident,
                             start=True, stop=True)

        # constants ride the gpsimd queue: the two hwdge queues are left
        # free so the first query chunks start transferring immediately
        a_sb = singles.tile([128, 128], BF16)   # blockdiag(A, A)
        nc.gpsimd.dma_start(a_sb, t["a2"][:])
        m_sb = singles.tile([128, 5, E], BF16)   # loaded later, needed by post
        btot_rep = singles.tile([128, E], F32)   # broadcast-loaded later

        # mask -> additive bias, [128, NKC] with k = kt*128 + p
        mask_i = singles.tile([128, NKC], I32)
        nc.gpsimd.dma_start(mask_i, t["mask"][:].rearrange("(kt p) -> p kt", p=128))
        mask_f = singles.tile([128, NKC], F32)
        nc.vector.tensor_copy(mask_f, mask_i)
        mbias = singles.tile([128, NKC], F32)
        # (mask - 1) * (-MASK_BIAS):  mask=0 -> MASK_BIAS, mask=1 -> 0
        nc.vector.tensor_scalar(out=mbias, in0=mask_f, scalar1=1.0,
                                scalar2=-MASK_BIAS, op0=sub, op1=mult)

        # ---- persistent tensors ----
        qt = singles.tile([128, 4, SQ], BF16)    # query^T: e=fc*128+p
        kt = singles.tile([128, 4, SK], BF16)    # key^T
        vt = singles.tile([128, NKC, H, D + 1], BF16)  # value + ones col
        xqa = singles.tile([128, 4, SQ], BF16)   # (Xq @ A)^T per head
        # normalized (attn @ V)^T: head pair layout, odd heads at base 64
        avt = singles.tile([128, 4, SQ], BF16)

        # ---- chunk loader: load f32, cast bf16, PE-transpose into dst ----
        def load_transposed_chunk(src, dst, c, queue, queue2=None):
            raw = stage.tile([128, E], F32, tag="ld")
            if queue2 is not None:
                # split across both hwdge queues: halves transfer in
                # parallel, halving the chunk's arrival latency
                queue.dma_start(raw[:, 0:E // 2],
                                src[c * 128:(c + 1) * 128, 0:E // 2])
                queue2.dma_start(raw[:, E // 2:E],
                                 src[c * 128:(c + 1) * 128, E // 2:E])
            else:
                queue.dma_start(raw, src[c * 128:(c + 1) * 128, :])
            cast = stage.tile([128, E], BF16, tag="cast")
            nc.vector.tensor_copy(cast, raw)
            tp = p_sc.tile([128, 4, 128], BF16, tag="sc")
            for eg in range(4):
                nc.tensor.transpose(tp[:, eg, :],
                                    cast[:, eg * 128:(eg + 1) * 128], ident)
            nc.vector.tensor_copy(dst[:, :, c * 128:(c + 1) * 128], tp)

        # ---- Q: load + transpose, then the A-projection per head pair ----
        # alternate full-chunk loads over both hwdge queues (the transfers
        # are descriptor/latency-bound: halving them into column splits
        # doubles descriptor overhead and is slower)
        qqs = [nc.sync, nc.scalar]
        for c in range(NQC):
            load_transposed_chunk(t["query"][:], qt, c, qqs[c % 2])
        for fc in range(4):
            for g in range(NQG):
                gsl = slice(g * QGS, (g + 1) * QGS)
                ps = p_sc.tile([128, QGS], F32, tag="sc")
                nc.tensor.matmul(ps, lhsT=a_sb, rhs=qt[:, fc, gsl],
                                 start=True, stop=True)
                nc.vector.tensor_copy(xqa[:, fc, gsl], ps)

        # ---- optional exact bq correction: per-(h,k) additive bias ----
        # scores^T gains (Xk_h @ (Wk^T bq / 8))[k], constant over q.
        if has_qbias:
            GW = 512 if SK % 512 == 0 else 384
            for c in range(NKC):
                load_transposed_chunk(t["key"][:], kt, c, nc.gpsimd)
            w2 = singles.tile([128, 1], BF16)
            nc.sync.dma_start(w2, t["w2"][:])
            hbias = []
            for h in range(H):
                bp, fc = 64 * (h % 2), h // 2
                row = small.tile([1, SK], F32, tag="hb_row")
                for g in range(SK // GW):
                    ps = p_misc.tile([128, GW], F32, tag="ps")
                    nc.tensor.matmul(ps[0:1, :], lhsT=w2[bp:bp + 64, :],
                                     rhs=kt[bp:bp + 64, fc, g * GW:(g + 1) * GW],
                                     start=True, stop=True)
                    nc.vector.tensor_copy(row[:, g * GW:(g + 1) * GW],
                                          ps[0:1, :])
                hb = singles.tile([128, NKC], F32, name=f"hbias{h}")
                nc.gpsimd.dma_start(hb, row[0, :].rearrange("(kt p) -> p kt",
                                                            p=128))
                nc.vector.tensor_tensor(out=hb, in0=hb, in1=mbias, op=add)
                hbias.append(hb)
        else:
            hbias = [mbias] * H

        # ---- per head-pair: scores^T -> exp -> AV^T, row-group alternated ---
        # K-chunk transposes are folded into pair 0; attn@V matmuls of the
        # previous pair's heads are folded into later pairs for PE smoothness.
        av_state = {}

        def av_alloc(h, pool=None, tag="ps"):
            pool = pool or p_misc
            av_state[h] = [pool.tile([128, QGS], F32, tag=tag,
                                     name=f"av{h}g{g}") for g in range(NQG)]

        def av_mm(h, exA, exB, c):
            ex, ci = (exA, c) if c < H0 else (exB, c - H0)
            for g in range(NQG):
                nc.tensor.matmul(av_state[h][g][0:D + 1, :],
                                 lhsT=vt[:, c, h, :],
                                 rhs=ex[:, ci, g * QGS:(g + 1) * QGS],
                                 start=(c == 0), stop=(c == NKC - 1))

        # Normalization runs in two stages so the expensive DVE reciprocal
        # (a ucode op, ~6.5ns/elem regardless of partition count) is batched
        # over a whole head pair: av_copy(h) drains PSUM per head and DMAs
        # its denominator rows into a gather tile; av_finish(b) reciprocals
        # all 2*NQG rows of the pair in ONE DVE op, broadcasts each across
        # partitions with a rank-1 f32r matmul, and multiplies.
        avsb_t = {}
        den_t = {}

        # den gather tiles for batches 0..2 (stream_shuffle maps
        # out[j+32k] = in[mask[j]+32k], so each (h,g) denominator row is
        # replicated at partitions i and i+32); memsets run upfront on an
        # idle DVE instead of crowding the pair boundaries
        for b in range(3):
            den_t[b] = small.tile([64, QGS], F32, tag="den", bufs=3,
                                  name=f"den{b}")
            nc.vector.memset(den_t[b], 1.0)

        def av_copy(h, gather_den=True):
            b = h // 2
            for g in range(NQG):
                ps = av_state[h][g]
                i = (h % 2) * NQG + g
                avsb = small.tile([128, QGS], F32R, tag="avsb", bufs=4,
                                  name=f"avsb{h}g{g}")
                nc.vector.tensor_copy(avsb[0:D + 1, :], ps[0:D + 1, :])
                if gather_den:
                    # (hwdge queues regress here: the scalar queue is issued
                    # by the ACT sequencer, which is saturated with EXP work)
                    den_row = avsb[D:D + 1, :].bitcast(F32)
                    nc.gpsimd.dma_start(den_t[b][i:i + 1, :], den_row)
                    nc.gpsimd.dma_start(den_t[b][i + 32:i + 33, :], den_row)
                avsb_t[(h, g)] = avsb
            del av_state[h]

        rec_t = {}

        def _norm_head(b, h, rec):
            # rec rows for head h hold the reciprocal'd denominators at
            # partitions i and i+32; stream_shuffle (DVE partition crossbar)
            # broadcasts them to all 64 partitions, then multiply. The whole
            # chain is DVE+DMA only, so the PE stream never stalls on it.
            fc = h // 2
            for g in range(NQG):
                i = (h % 2) * NQG + g
                avsb = avsb_t.pop((h, g))
                bc = small.tile([64, QGS], F32, tag="bc", bufs=4,
                                name=f"bc{b}i{i}")
                nc.vector.stream_shuffle(bc, rec, [i] * 32)
                gsl = slice(g * QGS, (g + 1) * QGS)
                if h % 2 == 0:
                    nc.vector.tensor_tensor(out=avt[0:D, fc, gsl],
                                            in0=avsb[0:D, :].bitcast(F32),
                                            in1=bc, op=mult)
                else:
                    avtmp = small.tile([64, QGS], BF16, tag="avtmp")
                    nc.vector.tensor_tensor(out=avtmp,
                                            in0=avsb[0:D, :].bitcast(F32),
                                            in1=bc, op=mult)
                    nc.gpsimd.dma_start(avt[64:64 + D, fc, gsl], avtmp)

        def av_finish(b):
            # one batched reciprocal for the whole pair, then normalize the
            # odd head first so its extra SBUF->SBUF partition-hop DMA
            # overlaps the even head's direct writes
            rec = small.tile([64, QGS], F32, tag="rec", bufs=2,
                             name=f"rec{b}")
            nc.vector.reciprocal(rec, den_t[b])
            _norm_head(b, 2 * b + 1, rec)
            _norm_head(b, 2 * b, rec)
            del den_t[b]

        def av_finish_part(b, h):
            # per-head half-batch: reciprocal the whole gather tile as soon
            # as this head's rows are in (the other head's rows are still
            # memset 1.0), so the first head's norm chain hides under the
            # second head's AV matmuls on PE
            rec = small.tile([64, QGS], F32, tag="rec", bufs=2,
                             name=f"rec{b}h{h}")
            nc.vector.reciprocal(rec, den_t[b])
            _norm_head(b, h, rec)

        def emit_half(p, half, av_head=None, av_tiles=None, ktrans=False,
                      norm_cb=None):
            h0, h1 = 2 * p, 2 * p + 1
            hsz, base = (H0, 0) if half == 0 else (H1, H0)
            # norm work for earlier heads is emitted BEFORE this half's
            # av_alloc: its pb tiles then reuse p_misc ring slots whose
            # drain copies were emitted in the same callback, keeping the
            # per-engine FIFOs acyclic (emitting it later deadlocks)
            if norm_cb is not None:
                norm_cb()
            # one tag -> one ring of bufs; allocate max-half shape always
            exs = [expp.tile([128, H0, SQ], BF16, tag="exp",
                             name=f"exp_{h}_{half}") for h in (h0, h1)]
            if av_head is not None and av_head not in av_state:
                av_alloc(av_head)
            av_sched = _split_even(NKC, hsz)
            for kt_l in range(hsz):
                kt_i = base + kt_l
                if ktrans:
                    load_transposed_chunk(t["key"][:], kt, kt_i, nc.gpsimd)
                pss = [p_sc.tile([128, SQ], F32, tag="sc", name=f"sc{j}")
                       for j in range(2)]
                # j outer, g inner: consecutive matmuls share lhsT, halving
                # LDWEIGHTS swaps, and pss[0] completes before pss[1] starts
                # so its exp overlaps the second head's matmuls
                for j, h in enumerate((h0, h1)):
                    bp, fc = 64 * (h % 2), h // 2
                    for g in range(NQG):
                        nc.tensor.matmul(
                            pss[j][:, g * QGS:(g + 1) * QGS],
                            lhsT=kt[bp:bp + 64, fc,
                                    kt_i * 128:(kt_i + 1) * 128],
                            rhs=xqa[bp:bp + 64, fc, g * QGS:(g + 1) * QGS],
                            start=True, stop=True)
                if av_head is not None:
                    for c in av_sched[kt_l]:
                        av_mm(av_head, av_tiles[0], av_tiles[1], c)
                for j, h in enumerate((h0, h1)):
                    nc.scalar.activation(out=exs[j][:, kt_l, :], in_=pss[j],
                                         func=mybir.ActivationFunctionType.Exp,
                                         bias=hbias[h][:, kt_i:kt_i + 1],
                                         scale=1.0)
            return exs

        halves = {}

        def set_halves(p, half, exs):
            halves[(2 * p, half)], halves[(2 * p + 1, half)] = exs

        def pop_head(h):
            return halves.pop((h, 0)), halves.pop((h, 1))

        set_halves(0, 0, emit_half(0, 0, ktrans=not has_qbias))
        set_halves(0, 1, emit_half(0, 1, ktrans=not has_qbias))
        # V: load + cast while pair-1 scores run (needed first by AV(h0))
        nc.gpsimd.dma_start(btot_rep,
                            t["btot"][:][None, :].to_broadcast([128, E]))
        nc.vector.memset(vt[:, :, :, D:D + 1], 1.0)
        for c in range(NKC):
            raw = stage.tile([128, E], F32, tag="ld")
            nc.sync.dma_start(raw, t["value"][c * 128:(c + 1) * 128, :])
            nc.vector.tensor_copy(vt[:, c, :, 0:D],
                                  raw.rearrange("p (h d) -> p h d", h=H))
        norm_cbs = {
            (1, 1): lambda: av_copy(0),
            (2, 0): lambda: (av_copy(1), av_finish(0)),
            (2, 1): lambda: av_copy(2),
            (3, 0): lambda: (av_copy(3), av_finish(1)),
            (3, 1): lambda: av_copy(4),
        }
        for p in range(1, H // 2):
            hh = 2 * (p - 1)
            ta, tb = pop_head(hh)
            set_halves(p, 0, emit_half(p, 0, av_head=hh, av_tiles=(ta, tb),
                                       norm_cb=norm_cbs.get((p, 0))))
            ta, tb = pop_head(hh + 1)
            set_halves(p, 1, emit_half(p, 1, av_head=hh + 1,
                                       av_tiles=(ta, tb),
                                       norm_cb=norm_cbs.get((p, 1))))
        nc.sync.dma_start(m_sb, t["m2"][:])
        t6, t7 = pop_head(H - 2), pop_head(H - 1)
        av_copy(5)
        av_finish(2)

        # ---- output projection pass 1: head slots 0..2 are done; fold them
        # (plus the output bias, so pass 2 needs only one add) into a f32
        # accumulator while the last exps drain on ACT. Head pairs sit at
        # partition bases 0/64 of avt and m_sb, so a single 128-contraction
        # matmul sums both heads of a slot.
        out_acc = singles.tile([128, NQC, E], F32)
        for q_i in range(SQ // 128):
            qs = slice(q_i * 128, (q_i + 1) * 128)
            psA = p_misc.tile([128, E], F32, tag="ps")
            for j in range(3):
                nc.tensor.matmul(psA, lhsT=avt[:, j, qs], rhs=m_sb[:, j, :],
                                 start=(j == 0), stop=(j == 2))
            nc.vector.tensor_tensor(out=out_acc[:, q_i, :], in0=psA,
                                    in1=btot_rep, op=add)
        # ---- last pair: normalize AFTER the output projection ----
        # The out-proj result has q on partitions, so 1/den becomes a
        # per-PARTITION scale: transpose the denominator rows into columns
        # on PE (16 tiny transposes), reciprocal them all in ONE cheap DVE
        # op ([128,16], free-size-bound: ~0.1us vs 2x3.35us), and fold the
        # scale + accumulate into fused scalar_tensor_tensor ops. No
        # serial reciprocal, no shuffles, no partition-hop DMA in the tail.
        av_alloc(H - 2)
        for c in range(NKC):
            av_mm(H - 2, t6[0], t6[1], c)
        av_copy(H - 2, gather_den=False)
        av_alloc(H - 1)
        for c in range(NKC):
            av_mm(H - 1, t7[0], t7[1], c)
        av_copy(H - 1, gather_den=False)

        # unnormalized bf16 AV for the last pair, each head at base 0
        avu = {}
        for h in (H - 2, H - 1):
            for g in range(NQG):
                avsb = avsb_t.pop((h, g))
                u = small.tile([64, QGS], BF16, tag="avu", bufs=4,
                               name=f"avu{h}g{g}")
                nc.vector.tensor_copy(u, avsb[0:D, :].bitcast(F32))
                avu[(h, g)] = (u, avsb)

        # denominator rows -> columns: rank-1 matmuls (out[i,0] =
        # row[0,i] * 1.0; the 1x1 identity slice sits at base partition 64
        # to match the row's base)
        dcol = p_sc.tile([128, SQ], F32, tag="sc")
        for hi, h in enumerate((H - 2, H - 1)):
            for g in range(NQG):
                avsb = avu[(h, g)][1]
                for c in range(4):
                    idx = hi * 8 + g * 4 + c
                    nc.tensor.matmul(
                        dcol[:, idx:idx + 1],
                        lhsT=avsb[D:D + 1, c * 128:(c + 1) * 128].bitcast(F32),
                        rhs=identf[64:65, 64:65],
                        start=True, stop=True)
        rcol = small.tile([128, 16], F32)
        nc.vector.reciprocal(rcol, dcol[:, 0:16])

        # per-head out-proj + fused scale/accumulate, then store
        for q_i in range(SQ // 128):
            qs = slice(q_i * 128, (q_i + 1) * 128)
            g, c = q_i // 4, q_i % 4
            csl = slice(c * 128, (c + 1) * 128)
            ps6 = p_misc.tile([128, E], F32, tag="ps")
            nc.tensor.matmul(ps6, lhsT=avu[(H - 2, g)][0][:, csl],
                             rhs=m_sb[0:D, 3, :], start=True, stop=True)
            ps7 = p_misc.tile([128, E], F32, tag="ps")
            nc.tensor.matmul(ps7, lhsT=avu[(H - 1, g)][0][:, csl],
                             rhs=m_sb[0:D, 4, :], start=True, stop=True)
            ob = outp.tile([128, E], F32, tag="ob", bufs=4)
            nc.vector.scalar_tensor_tensor(
                out=ob, in0=ps6, scalar=rcol[:, q_i:q_i + 1],
                in1=out_acc[:, q_i, :], op0=mult, op1=add)
            ob2 = outp.tile([128, E], F32, tag="ob2", bufs=4)
            nc.vector.scalar_tensor_tensor(
                out=ob2, in0=ps7, scalar=rcol[:, 8 + q_i:9 + q_i],
                in1=ob, op0=mult, op1=add)
            nc.sync.dma_start(t["out"][qs, :], ob2)


def build_module(SQ=SQ, SK=SK_PAD, has_qbias=False):
    nc = bacc.Bacc()
    t = {
        "query": nc.dram_tensor("query", [SQ, E], F32, kind="ExternalInput"),
        "key": nc.dram_tensor("key", [SK, E], F32, kind="ExternalInput"),
        "value": nc.dram_tensor("value", [SK, E], F32, kind="ExternalInput"),
        "mask": nc.dram_tensor("mask", [SK], I32, kind="ExternalInput"),
        "a2": nc.dram_tensor("a2", [128, 128], BF16, kind="ExternalInput"),
        "m2": nc.dram_tensor("m2", [128, 5, E], BF16, kind="ExternalInput"),
        "btot": nc.dram_tensor("btot", [E], F32, kind="ExternalInput"),
        "out": nc.dram_tensor("out", [SQ, E], F32, kind="ExternalOutput"),
    }
    if has_qbias:
        t["w2"] = nc.dram_tensor("w2", [128, 1], BF16, kind="ExternalInput")
    with tile.TileContext(nc) as tc:
        _emit(tc, t, SQ, SK, has_qbias)
    nc.compile()
    return nc


_MODULE_CACHE = {}


def _get_module(SQ, SK, has_qbias):
    key = (SQ, SK, has_qbias)
    if key not in _MODULE_CACHE:
        _MODULE_CACHE[key] = build_module(SQ, SK, has_qbias)
    return _MODULE_CACHE[key]


def _fold_weights(Wq, Wk, Wv, Wo, bv, bo):
    Wq, Wk, Wv, Wo = (np.asarray(w, np.float64) for w in (Wq, Wk, Wv, Wo))
    A = (Wq.T @ Wk) / np.sqrt(np.float64(D))
    a2 = np.zeros((128, 128), np.float64)   # blockdiag: one matmul per pair
    a2[0:D, 0:D] = A
    a2[D:2 * D, D:2 * D] = A
    a2 = a2.astype(ml_dtypes.bfloat16)
    Ms = [Wv.T @ Wo[:, h * D:(h + 1) * D].T for h in range(H)]
    # head-pair packing: head h at partitions 64*(h%2) .. +64, free slot
    # h//2; slot 4 repeats M7 at base 0 for the per-head tail projection
    m2 = np.zeros((128, 5, E), np.float64)
    for h in range(H):
        m2[64 * (h % 2):64 * (h % 2) + D, h // 2, :] = Ms[h]
    m2[0:D, 4, :] = Ms[H - 1]
    m2 = m2.astype(ml_dtypes.bfloat16)
    btot = (np.asarray(bo, np.float64)
            + Wo @ np.tile(np.asarray(bv, np.float64), H)).astype(np.float32)
    return a2, m2, btot


def _run(inputs, trace=False):
    query = np.asarray(inputs["query"], np.float32)
    key = np.asarray(inputs["key"], np.float32)
    value = np.asarray(inputs["value"], np.float32)
    mask = np.asarray(inputs["mask"])
    a2, m2, btot = _fold_weights(inputs["Wq"], inputs["Wk"], inputs["Wv"],
                                 inputs["Wo"], inputs["bv"], inputs["bo"])
    bq = np.asarray(inputs["bq"], np.float64)
    bk = np.asarray(inputs["bk"], np.float64)  # noqa: F841  (cancels in softmax)
    has_qbias = bool(np.any(bq != 0))
    w2 = None
    if has_qbias:
        w2v = (np.asarray(inputs["Wk"], np.float64).T @ bq) / np.sqrt(float(D))
        w2 = np.concatenate([w2v, w2v]).reshape(128, 1).astype(ml_dtypes.bfloat16)

    n_batch, S = query.shape[0], query.shape[1]
    sq = S // 2

    # Per-batch key compaction: gather valid keys, zero-pad to SK_PAD.
    # exp(MASK_BIAS + s) == 0.0f for the pad rows, so this is exact.
    mask2d = mask.reshape(n_batch, S).astype(np.int32)
    n_valid = [int(np.count_nonzero(mask2d[n])) for n in range(n_batch)]
    if max(n_valid) <= SK_PAD and S >= SK_PAD:
        sk = SK_PAD
        key_c = np.zeros((n_batch, sk, E), np.float32)
        val_c = np.zeros((n_batch, sk, E), np.float32)
        mask_c = np.zeros((n_batch, sk), np.int32)
        for n in range(n_batch):
            idx = np.nonzero(mask2d[n])[0]
            key_c[n, :len(idx)] = key[n][idx]
            val_c[n, :len(idx)] = value[n][idx]
            mask_c[n, :len(idx)] = 1
    else:
        sk = S
        key_c, val_c, mask_c = key, value, mask2d

    nc = _get_module(sq, sk, has_qbias)

    in_maps = []
    for c in range(N_CORES):
        n, qh = divmod(c, 2)
        m = {
            "query": np.ascontiguousarray(query[n, qh * sq:(qh + 1) * sq, :]),
            "key": np.ascontiguousarray(key_c[n]),
            "value": np.ascontiguousarray(val_c[n]),
            "mask": np.ascontiguousarray(mask_c[n]),
            "a2": a2, "m2": m2, "btot": btot,
        }
        if has_qbias:
            m["w2"] = w2
        in_maps.append(m)

    res = run_bass_kernel_spmd(nc, in_maps, core_ids=list(range(N_CORES)),
                               trace=trace)
    out = np.empty((n_batch, S, E), np.float32)
    for c, r in enumerate(res.results):
        n, qh = divmod(c, 2)
        out[n, qh * sq:(qh + 1) * sq, :] = r["out"]
    return out, res


def kernel(**inputs) -> np.ndarray:
    out, _ = _run(inputs, trace=False)
    return out


# revision 50
# speedup vs baseline: 1.1829x; 1.1829x over previous
# Multi-head attention (N=4, S=2048, E=512, H=8, D=64) on 8 NeuronCores.
#
# Sharding: core c -> (batch n = c//2, query half qh = c%2). Each core
# computes attention for its 1024 query rows against the valid keys of
# its batch, all 8 heads, and the full output projection for its rows, so
# outputs are disjoint and no collectives are needed.
#
# Host-side preprocessing (exact, done once per call):
#   - Key compaction: the mask is per-batch over keys; masked keys get
#     exp(-1e9/8) == 0.0f in the reference, so they contribute nothing.
#     K/V/mask are gathered to the valid keys and zero-padded to SK_PAD
#     rows (P(overflow) ~ 5.7 sigma per batch; falls back to the full
#     2048-key module if it ever happens).
#   - Weight folding (pure weight algebra):
#       A  = Wq^T @ Wk / 8          scores = (Xq @ A) @ Xk^T
#       M_h = Wv^T @ Wo[:, h]^T     out += (attn @ Xv_h) @ M_h
#       btot = bo + Wo @ tile(bv,8) exact because attention rows sum to 1
#       bk-term cancels in softmax; bq-term via per-k bias (zero here).
#     A is stored block-diagonally [128,128] so one matmul projects a
#     head pair at once.
#
# Device per core:
#   - PE-transpose Q,K chunks (bf16) to [e, s] layout
#   - XqA^T = blockdiag(A)^T-proj of Q^T per head pair
#   - scores^T[k,q] = K_h^T.T @ XqA_h^T  (PSUM f32)
#   - exp on ACT straight from PSUM with per-partition mask bias -> bf16
#   - AV^T = V_aug.T @ exp^T with a ones column in V giving softmax
#     denominators as row 64; reciprocal on ACT (same act table as Exp),
#     broadcast across partitions with a rank-1 f32r matmul, multiply.
#   - out[q,:] = sum_h AVT_h.T @ M_h (+btot): head pairs at partition
#     bases 0/64 make this a single 128-contraction matmul per slot.

import numpy as np
import ml_dtypes

import concourse.bass as bass
import concourse.tile as tile
from concourse import bacc, mybir
from concourse.bass_utils import run_bass_kernel_spmd
from concourse.masks import make_identity

F32 = mybir.dt.float32
BF16 = mybir.dt.bfloat16
I32 = mybir.dt.int32

H = 8
D = 64
E = 512
N_CORES = 8
FULL_N, FULL_S = 4, 2048
SQ = 1024            # per-core query rows
SK_PAD = 1152        # compacted+padded key rows (9 chunks of 128)
MASK_BIAS = -1.25e8  # == -1e9 / sqrt(64), applied pre-softmax

# Engine facts probed on this runtime: Pool partition_broadcast and custom-DVE
# ops produce garbage / fail ISA checks; DVE divide and Pool
# scalar_tensor_tensor are rejected by walrus; DVE fp8 converts and fp8
# DoubleRow matmuls DO work; stream_shuffle semantics: out[j+32k]=in[mask[j]+32k].


def _split_even(n_items, n_steps):
    """Distribute range(n_items) into n_steps contiguous chunks, sizes
    as even as possible (larger first)."""
    out, base = [], 0
    for s in range(n_steps):
        take = (n_items - base + (n_steps - s - 1)) // (n_steps - s)
        out.append(list(range(base, base + take)))
        base += take
    return out


def _emit(tc, t, SQ, SK, has_qbias):
    nc = tc.nc
    NQC = SQ // 128           # query chunks (transpose granularity)
    NKC = SK // 128           # key chunks
    QGS = min(512, SQ)        # q group size for matmul free dim
    NQG = SQ // QGS
    H0 = (NKC + 1) // 2       # k-chunks in half 0 (exp half-tile)
    H1 = NKC - H0
    F32R = mybir.dt.float32r
    sub, mult, add = (mybir.AluOpType.subtract, mybir.AluOpType.mult,
                      mybir.AluOpType.add)

    with (
        tc.tile_pool(name="singles", bufs=1) as singles,
        tc.tile_pool(name="stage", bufs=6) as stage,
        tc.tile_pool(name="expp", bufs=7) as expp,
        tc.tile_pool(name="small", bufs=2) as small,
        tc.tile_pool(name="outp", bufs=2) as outp,
        tc.tile_pool(name="p_sc", bufs=2, space="PSUM") as p_sc,
        tc.tile_pool(name="p_misc", bufs=4, space="PSUM") as p_misc,
    ):
        # ---- constants / weights ----
        ident = singles.tile([128, 128], BF16)
        make_identity(nc, ident)
        identf = singles.tile([128, 128], F32)
        nc.vector.tensor_copy(identf, ident)

        # Dummy matmuls: the PE p-state ramps to max (2.4GHz) only after 3us
        # of CONTINUOUS execution, and any idle gap resets the ramp (the
        # next ~3us then run at 1.2GHz). Burst enough to cover the DMA wait
        # for the first query chunk, and sprinkle more later as gap filler
        # during the load-paced warmup phases.
        warm = p_misc.tile([128, 128], F32, tag="ps")

        def pe_warm_fill(n):
            for _ in range(n):
                nc.tensor.matmul(warm, lhsT=ident, rhs=ident,
                                 start=True, stop=True)

        pe_warm_fill(48)

        # constants ride the gpsimd queue: the two hwdge queues are left
        # free so the first query chunks start transferring immediately
        a_sb = singles.tile([128, 128], BF16)   # blockdiag(A, A)
        nc.gpsimd.dma_start(a_sb, t["a2"][:])
        m_sb = singles.tile([128, 5, E], BF16)   # loaded later, needed by post
        btot_rep = singles.tile([128, E], F32)   # broadcast-loaded later

        # mask -> additive bias, [128, NKC] with k = kt*128 + p
        mask_i = singles.tile([128, NKC], I32)
        nc.gpsimd.dma_start(mask_i, t["mask"][:].rearrange("(kt p) -> p kt", p=128))
        mask_f = singles.tile([128, NKC], F32)
        nc.vector.tensor_copy(mask_f, mask_i)
        mbias = singles.tile([128, NKC], F32)
        # (mask - 1) * (-MASK_BIAS):  mask=0 -> MASK_BIAS, mask=1 -> 0
        nc.vector.tensor_scalar(out=mbias, in0=mask_f, scalar1=1.0,
                                scalar2=-MASK_BIAS, op0=sub, op1=mult)

        # ---- persistent tensors ----
        qt = singles.tile([128, 4, SQ], BF16)    # query^T: e=fc*128+p
        kt = singles.tile([128, 4, SK], BF16)    # key^T
        vt = singles.tile([128, NKC, H, D + 1], BF16)  # value + ones col
        xqa = singles.tile([128, 4, SQ], BF16)   # (Xq @ A)^T per head
        # normalized (attn @ V)^T: head pair layout, odd heads at base 64
        avt = singles.tile([128, 4, SQ], BF16)

        # ---- chunk loader: load f32, cast bf16, PE-transpose into dst ----
        def load_transposed_chunk(src, dst, c, queue, queue2=None):
            raw = stage.tile([128, E], F32, tag="ld")
            if queue2 is not None:
                # split across both hwdge queues: halves transfer in
                # parallel, halving the chunk's arrival latency
                queue.dma_start(raw[:, 0:E // 2],
                                src[c * 128:(c + 1) * 128, 0:E // 2])
                queue2.dma_start(raw[:, E // 2:E],
                                 src[c * 128:(c + 1) * 128, E // 2:E])
            else:
                queue.dma_start(raw, src[c * 128:(c + 1) * 128, :])
            cast = stage.tile([128, E], BF16, tag="cast")
            nc.vector.tensor_copy(cast, raw)
            tp = p_sc.tile([128, 4, 128], BF16, tag="sc")
            for eg in range(4):
                nc.tensor.transpose(tp[:, eg, :],
                                    cast[:, eg * 128:(eg + 1) * 128], ident)
            nc.vector.tensor_copy(dst[:, :, c * 128:(c + 1) * 128], tp)

        # ---- Q: load + transpose, then the A-projection per head pair ----
        # alternate full-chunk loads over both hwdge queues (the transfers
        # are descriptor/latency-bound: halving them into column splits
        # doubles descriptor overhead and is slower)
        qqs = [nc.sync, nc.scalar]
        for c in range(NQC):
            load_transposed_chunk(t["query"][:], qt, c, qqs[c % 2])
            pe_warm_fill(10)
        for fc in range(4):
            for g in range(NQG):
                gsl = slice(g * QGS, (g + 1) * QGS)
                ps = p_sc.tile([128, QGS], F32, tag="sc")
                nc.tensor.matmul(ps, lhsT=a_sb, rhs=qt[:, fc, gsl],
                                 start=True, stop=True)
                nc.vector.tensor_copy(xqa[:, fc, gsl], ps)

        # ---- optional exact bq correction: per-(h,k) additive bias ----
        # scores^T gains (Xk_h @ (Wk^T bq / 8))[k], constant over q.
        if has_qbias:
            GW = 512 if SK % 512 == 0 else 384
            for c in range(NKC):
                load_transposed_chunk(t["key"][:], kt, c, nc.gpsimd)
            w2 = singles.tile([128, 1], BF16)
            nc.sync.dma_start(w2, t["w2"][:])
            hbias = []
            for h in range(H):
                bp, fc = 64 * (h % 2), h // 2
                row = small.tile([1, SK], F32, tag="hb_row")
                for g in range(SK // GW):
                    ps = p_misc.tile([128, GW], F32, tag="ps")
                    nc.tensor.matmul(ps[0:1, :], lhsT=w2[bp:bp + 64, :],
                                     rhs=kt[bp:bp + 64, fc, g * GW:(g + 1) * GW],
                                     start=True, stop=True)
                    nc.vector.tensor_copy(row[:, g * GW:(g + 1) * GW],
                                          ps[0:1, :])
                hb = singles.tile([128, NKC], F32, name=f"hbias{h}")
                nc.gpsimd.dma_start(hb, row[0, :].rearrange("(kt p) -> p kt",
                                                            p=128))
                nc.vector.tensor_tensor(out=hb, in0=hb, in1=mbias, op=add)
                hbias.append(hb)
        else:
            hbias = [mbias] * H

        # ---- per head-pair: scores^T -> exp -> AV^T, row-group alternated ---
        # K-chunk transposes are folded into pair 0; attn@V matmuls of the
        # previous pair's heads are folded into later pairs for PE smoothness.
        av_state = {}

        def av_alloc(h, pool=None, tag="ps"):
            pool = pool or p_misc
            av_state[h] = [pool.tile([128, QGS], F32, tag=tag,
                                     name=f"av{h}g{g}") for g in range(NQG)]

        def av_mm(h, exA, exB, c):
            ex, ci = (exA, c) if c < H0 else (exB, c - H0)
            for g in range(NQG):
                nc.tensor.matmul(av_state[h][g][0:D + 1, :],
                                 lhsT=vt[:, c, h, :],
                                 rhs=ex[:, ci, g * QGS:(g + 1) * QGS],
                                 start=(c == 0), stop=(c == NKC - 1))

        # Normalization runs in two stages so the expensive DVE reciprocal
        # (a ucode op, ~6.5ns/elem regardless of partition count) is batched
        # over a whole head pair: av_copy(h) drains PSUM per head and DMAs
        # its denominator rows into a gather tile; av_finish(b) reciprocals
        # all 2*NQG rows of the pair in ONE DVE op, broadcasts each across
        # partitions with a rank-1 f32r matmul, and multiplies.
        avsb_t = {}
        den_t = {}

        # den gather tiles for batches 0..2 (stream_shuffle maps
        # out[j+32k] = in[mask[j]+32k], so each (h,g) denominator row is
        # replicated at partitions i and i+32); memsets run upfront on an
        # idle DVE instead of crowding the pair boundaries
        for b in range(3):
            den_t[b] = small.tile([64, QGS], F32, tag="den", bufs=3,
                                  name=f"den{b}")
            nc.vector.memset(den_t[b], 1.0)

        def av_copy(h, gather_den=True):
            b = h // 2
            for g in range(NQG):
                ps = av_state[h][g]
                i = (h % 2) * NQG + g
                avsb = small.tile([128, QGS], F32R, tag="avsb", bufs=4,
                                  name=f"avsb{h}g{g}")
                nc.vector.tensor_copy(avsb[0:D + 1, :], ps[0:D + 1, :])
                if gather_den:
                    # (hwdge queues regress here: the scalar queue is issued
                    # by the ACT sequencer, which is saturated with EXP work)
                    den_row = avsb[D:D + 1, :].bitcast(F32)
                    nc.gpsimd.dma_start(den_t[b][i:i + 1, :], den_row)
                    nc.gpsimd.dma_start(den_t[b][i + 32:i + 33, :], den_row)
                avsb_t[(h, g)] = avsb
            del av_state[h]

        rec_t = {}

        def _norm_head(b, h, rec):
            # rec rows for head h hold the reciprocal'd denominators at
            # partitions i and i+32; stream_shuffle (DVE partition crossbar)
            # broadcasts them to all 64 partitions, then multiply. The whole
            # chain is DVE+DMA only, so the PE stream never stalls on it.
            fc = h // 2
            for g in range(NQG):
                i = (h % 2) * NQG + g
                avsb = avsb_t.pop((h, g))
                bc = small.tile([64, QGS], F32, tag="bc", bufs=4,
                                name=f"bc{b}i{i}")
                nc.vector.stream_shuffle(bc, rec, [i] * 32)
                gsl = slice(g * QGS, (g + 1) * QGS)
                if h % 2 == 0:
                    nc.vector.tensor_tensor(out=avt[0:D, fc, gsl],
                                            in0=avsb[0:D, :].bitcast(F32),
                                            in1=bc, op=mult)
                else:
                    avtmp = small.tile([64, QGS], BF16, tag="avtmp")
                    nc.vector.tensor_tensor(out=avtmp,
                                            in0=avsb[0:D, :].bitcast(F32),
                                            in1=bc, op=mult)
                    nc.gpsimd.dma_start(avt[64:64 + D, fc, gsl], avtmp)

        def av_finish(b):
            # one batched reciprocal for the whole pair, then normalize the
            # odd head first so its extra SBUF->SBUF partition-hop DMA
            # overlaps the even head's direct writes
            rec = small.tile([64, QGS], F32, tag="rec", bufs=2,
                             name=f"rec{b}")
            nc.vector.reciprocal(rec, den_t[b])
            _norm_head(b, 2 * b + 1, rec)
            _norm_head(b, 2 * b, rec)
            del den_t[b]

        def av_finish_part(b, h):
            # per-head half-batch: reciprocal the whole gather tile as soon
            # as this head's rows are in (the other head's rows are still
            # memset 1.0), so the first head's norm chain hides under the
            # second head's AV matmuls on PE
            rec = small.tile([64, QGS], F32, tag="rec", bufs=2,
                             name=f"rec{b}h{h}")
            nc.vector.reciprocal(rec, den_t[b])
            _norm_head(b, h, rec)

        def emit_half(p, half, av_head=None, av_tiles=None, ktrans=False,
                      norm_cb=None):
            h0, h1 = 2 * p, 2 * p + 1
            hsz, base = (H0, 0) if half == 0 else (H1, H0)
            # norm work for earlier heads is emitted BEFORE this half's
            # av_alloc: its pb tiles then reuse p_misc ring slots whose
            # drain copies were emitted in the same callback, keeping the
            # per-engine FIFOs acyclic (emitting it later deadlocks)
            if norm_cb is not None:
                norm_cb()
            # one tag -> one ring of bufs; allocate max-half shape always
            exs = [expp.tile([128, H0, SQ], BF16, tag="exp",
                             name=f"exp_{h}_{half}") for h in (h0, h1)]
            if av_head is not None and av_head not in av_state:
                av_alloc(av_head)
            av_sched = _split_even(NKC, hsz)
            for kt_l in range(hsz):
                kt_i = base + kt_l
                if ktrans:
                    load_transposed_chunk(t["key"][:], kt, kt_i, nc.gpsimd)
                pss = [p_sc.tile([128, SQ], F32, tag="sc", name=f"sc{j}")
                       for j in range(2)]
                # j outer, g inner: consecutive matmuls share lhsT, halving
                # LDWEIGHTS swaps, and pss[0] completes before pss[1] starts
                # so its exp overlaps the second head's matmuls
                for j, h in enumerate((h0, h1)):
                    bp, fc = 64 * (h % 2), h // 2
                    for g in range(NQG):
                        nc.tensor.matmul(
                            pss[j][:, g * QGS:(g + 1) * QGS],
                            lhsT=kt[bp:bp + 64, fc,
                                    kt_i * 128:(kt_i + 1) * 128],
                            rhs=xqa[bp:bp + 64, fc, g * QGS:(g + 1) * QGS],
                            start=True, stop=True)
                if av_head is not None:
                    for c in av_sched[kt_l]:
                        av_mm(av_head, av_tiles[0], av_tiles[1], c)
                for j, h in enumerate((h0, h1)):
                    nc.scalar.activation(out=exs[j][:, kt_l, :], in_=pss[j],
                                         func=mybir.ActivationFunctionType.Exp,
                                         bias=hbias[h][:, kt_i:kt_i + 1],
                                         scale=1.0)
            return exs

        halves = {}

        def set_halves(p, half, exs):
            halves[(2 * p, half)], halves[(2 * p + 1, half)] = exs

        def pop_head(h):
            return halves.pop((h, 0)), halves.pop((h, 1))

        set_halves(0, 0, emit_half(0, 0, ktrans=not has_qbias))
        pe_warm_fill(24)
        set_halves(0, 1, emit_half(0, 1, ktrans=not has_qbias))
        pe_warm_fill(16)
        # V: load + cast while pair-1 scores run (needed first by AV(h0))
        nc.gpsimd.dma_start(btot_rep,
                            t["btot"][:][None, :].to_broadcast([128, E]))
        nc.vector.memset(vt[:, :, :, D:D + 1], 1.0)
        for c in range(NKC):
            raw = stage.tile([128, E], F32, tag="ld")
            nc.sync.dma_start(raw, t["value"][c * 128:(c + 1) * 128, :])
            nc.vector.tensor_copy(vt[:, c, :, 0:D],
                                  raw.rearrange("p (h d) -> p h d", h=H))
        norm_cbs = {
            (1, 1): lambda: av_copy(0),
            (2, 0): lambda: (av_copy(1), av_finish(0)),
            (2, 1): lambda: av_copy(2),
            (3, 0): lambda: (av_copy(3), av_finish(1)),
            (3, 1): lambda: av_copy(4),
        }
        for p in range(1, H // 2):
            hh = 2 * (p - 1)
            ta, tb = pop_head(hh)
            set_halves(p, 0, emit_half(p, 0, av_head=hh, av_tiles=(ta, tb),
                                       norm_cb=norm_cbs.get((p, 0))))
            ta, tb = pop_head(hh + 1)
            set_halves(p, 1, emit_half(p, 1, av_head=hh + 1,
                                       av_tiles=(ta, tb),
                                       norm_cb=norm_cbs.get((p, 1))))
        nc.sync.dma_start(m_sb, t["m2"][:])
        t6, t7 = pop_head(H - 2), pop_head(H - 1)
        av_copy(5)
        av_finish(2)

        # ---- output projection pass 1: head slots 0..2 are done; fold them
        # (plus the output bias, so pass 2 needs only one add) into a f32
        # accumulator while the last exps drain on ACT. Head pairs sit at
        # partition bases 0/64 of avt and m_sb, so a single 128-contraction
        # matmul sums both heads of a slot.
        out_acc = singles.tile([128, NQC, E], F32)
        for q_i in range(SQ // 128):
            qs = slice(q_i * 128, (q_i + 1) * 128)
            psA = p_misc.tile([128, E], F32, tag="ps")
            for j in range(3):
                nc.tensor.matmul(psA, lhsT=avt[:, j, qs], rhs=m_sb[:, j, :],
                                 start=(j == 0), stop=(j == 2))
            nc.vector.tensor_tensor(out=out_acc[:, q_i, :], in0=psA,
                                    in1=btot_rep, op=add)
        # ---- last pair: normalize AFTER the output projection ----
        # The out-proj result has q on partitions, so 1/den becomes a
        # per-PARTITION scale: transpose the denominator rows into columns
        # on PE (16 tiny transposes), reciprocal them all in ONE cheap DVE
        # op ([128,16], free-size-bound: ~0.1us vs 2x3.35us), and fold the
        # scale + accumulate into fused scalar_tensor_tensor ops. No
        # serial reciprocal, no shuffles, no partition-hop DMA in the tail.
        av_alloc(H - 2)
        for c in range(NKC):
            av_mm(H - 2, t6[0], t6[1], c)
        av_copy(H - 2, gather_den=False)
        av_alloc(H - 1)
        for c in range(NKC):
            av_mm(H - 1, t7[0], t7[1], c)
        av_copy(H - 1, gather_den=False)

        # unnormalized bf16 AV for the last pair, each head at base 0
        avu = {}
        for h in (H - 2, H - 1):
            for g in range(NQG):
                avsb = avsb_t.pop((h, g))
                u = small.tile([64, QGS], BF16, tag="avu", bufs=4,
                               name=f"avu{h}g{g}")
                nc.vector.tensor_copy(u, avsb[0:D, :].bitcast(F32))
                avu[(h, g)] = (u, avsb)

        # denominator rows -> columns: rank-1 matmuls (out[i,0] =
        # row[0,i] * 1.0; the 1x1 identity slice sits at base partition 64
        # to match the row's base)
        dcol = p_sc.tile([128, SQ], F32, tag="sc")
        for hi, h in enumerate((H - 2, H - 1)):
            for g in range(NQG):
                avsb = avu[(h, g)][1]
                for c in range(4):
                    idx = hi * 8 + g * 4 + c
                    nc.tensor.matmul(
                        dcol[:, idx:idx + 1],
                        lhsT=avsb[D:D + 1, c * 128:(c + 1) * 128].bitcast(F32),
                        rhs=identf[64:65, 64:65],
                        start=True, stop=True)
        rcol = small.tile([128, 16], F32)
        nc.vector.reciprocal(rcol, dcol[:, 0:16])

        # per-head out-proj + fused scale/accumulate, then store
        for q_i in range(SQ // 128):
            qs = slice(q_i * 128, (q_i + 1) * 128)
            g, c = q_i // 4, q_i % 4
            csl = slice(c * 128, (c + 1) * 128)
            ps6 = p_misc.tile([128, E], F32, tag="ps")
            nc.tensor.matmul(ps6, lhsT=avu[(H - 2, g)][0][:, csl],
                             rhs=m_sb[0:D, 3, :], start=True, stop=True)
            ps7 = p_misc.tile([128, E], F32, tag="ps")
            nc.tensor.matmul(ps7, lhsT=avu[(H - 1, g)][0][:, csl],
                             rhs=m_sb[0:D, 4, :], start=True, stop=True)
            ob = outp.tile([128, E], F32, tag="ob", bufs=4)
            nc.vector.scalar_tensor_tensor(
                out=ob, in0=ps6, scalar=rcol[:, q_i:q_i + 1],
                in1=out_acc[:, q_i, :], op0=mult, op1=add)
            ob2 = outp.tile([128, E], F32, tag="ob2", bufs=4)
            nc.vector.scalar_tensor_tensor(
                out=ob2, in0=ps7, scalar=rcol[:, 8 + q_i:9 + q_i],
                in1=ob, op0=mult, op1=add)
            nc.sync.dma_start(t["out"][qs, :], ob2)


def build_module(SQ=SQ, SK=SK_PAD, has_qbias=False):
    nc = bacc.Bacc()
    t = {
        "query": nc.dram_tensor("query", [SQ, E], F32, kind="ExternalInput"),
        "key": nc.dram_tensor("key", [SK, E], F32, kind="ExternalInput"),
        "value": nc.dram_tensor("value", [SK, E], F32, kind="ExternalInput"),
        "mask": nc.dram_tensor("mask", [SK], I32, kind="ExternalInput"),
        "a2": nc.dram_tensor("a2", [128, 128], BF16, kind="ExternalInput"),
        "m2": nc.dram_tensor("m2", [128, 5, E], BF16, kind="ExternalInput"),
        "btot": nc.dram_tensor("btot", [E], F32, kind="ExternalInput"),
        "out": nc.dram_tensor("out", [SQ, E], F32, kind="ExternalOutput"),
    }
    if has_qbias:
        t["w2"] = nc.dram_tensor("w2", [128, 1], BF16, kind="ExternalInput")
    with tile.TileContext(nc) as tc:
        _emit(tc, t, SQ, SK, has_qbias)
    nc.compile()
    return nc


_MODULE_CACHE = {}


def _get_module(SQ, SK, has_qbias):
    key = (SQ, SK, has_qbias)
    if key not in _MODULE_CACHE:
        _MODULE_CACHE[key] = build_module(SQ, SK, has_qbias)
    return _MODULE_CACHE[key]


def _fold_weights(Wq, Wk, Wv, Wo, bv, bo):
    Wq, Wk, Wv, Wo = (np.asarray(w, np.float64) for w in (Wq, Wk, Wv, Wo))
    A = (Wq.T @ Wk) / np.sqrt(np.float64(D))
    a2 = np.zeros((128, 128), np.float64)   # blockdiag: one matmul per pair
    a2[0:D, 0:D] = A
    a2[D:2 * D, D:2 * D] = A
    a2 = a2.astype(ml_dtypes.bfloat16)
    Ms = [Wv.T @ Wo[:, h * D:(h + 1) * D].T for h in range(H)]
    # head-pair packing: head h at partitions 64*(h%2) .. +64, free slot
    # h//2; slot 4 repeats M7 at base 0 for the per-head tail projection
    m2 = np.zeros((128, 5, E), np.float64)
    for h in range(H):
        m2[64 * (h % 2):64 * (h % 2) + D, h // 2, :] = Ms[h]
    m2[0:D, 4, :] = Ms[H - 1]
    m2 = m2.astype(ml_dtypes.bfloat16)
    btot = (np.asarray(bo, np.float64)
            + Wo @ np.tile(np.asarray(bv, np.float64), H)).astype(np.float32)
    return a2, m2, btot


def _run(inputs, trace=False):
    query = np.asarray(inputs["query"], np.float32)
    key = np.asarray(inputs["key"], np.float32)
    value = np.asarray(inputs["value"], np.float32)
    mask = np.asarray(inputs["mask"])
    a2, m2, btot = _fold_weights(inputs["Wq"], inputs["Wk"], inputs["Wv"],
                                 inputs["Wo"], inputs["bv"], inputs["bo"])
    bq = np.asarray(inputs["bq"], np.float64)
    bk = np.asarray(inputs["bk"], np.float64)  # noqa: F841  (cancels in softmax)
    has_qbias = bool(np.any(bq != 0))
    w2 = None
    if has_qbias:
        w2v = (np.asarray(inputs["Wk"], np.float64).T @ bq) / np.sqrt(float(D))
        w2 = np.concatenate([w2v, w2v]).reshape(128, 1).astype(ml_dtypes.bfloat16)

    n_batch, S = query.shape[0], query.shape[1]
    sq = S // 2

    # Per-batch key compaction: gather valid keys, zero-pad to SK_PAD.
    # exp(MASK_BIAS + s) == 0.0f for the pad rows, so this is exact.
    mask2d = mask.reshape(n_batch, S).astype(np.int32)
    n_valid = [int(np.count_nonzero(mask2d[n])) for n in range(n_batch)]
    if max(n_valid) <= SK_PAD and S >= SK_PAD:
        sk = SK_PAD
        key_c = np.zeros((n_batch, sk, E), np.float32)
        val_c = np.zeros((n_batch, sk, E), np.float32)
        mask_c = np.zeros((n_batch, sk), np.int32)
        for n in range(n_batch):
            idx = np.nonzero(mask2d[n])[0]
            key_c[n, :len(idx)] = key[n][idx]
            val_c[n, :len(idx)] = value[n][idx]
            mask_c[n, :len(idx)] = 1
    else:
        sk = S
        key_c, val_c, mask_c = key, value, mask2d

    nc = _get_module(sq, sk, has_qbias)

    in_maps = []
    for c in range(N_CORES):
        n, qh = divmod(c, 2)
        m = {
            "query": np.ascontiguousarray(query[n, qh * sq:(qh + 1) * sq, :]),
            "key": np.ascontiguousarray(key_c[n]),
            "value": np.ascontiguousarray(val_c[n]),
            "mask": np.ascontiguousarray(mask_c[n]),
            "a2": a2, "m2": m2, "btot": btot,
        }
        if has_qbias:
            m["w2"] = w2
        in_maps.append(m)

    res = run_bass_kernel_spmd(nc, in_maps, core_ids=list(range(N_CORES)),
                               trace=trace)
    out = np.empty((n_batch, S, E), np.float32)
    for c, r in enumerate(res.results):
        n, qh = divmod(c, 2)
        out[n, qh * sq:(qh + 1) * sq, :] = r["out"]
    return out, res


def kernel(**inputs) -> np.ndarray:
    out, _ = _run(inputs, trace=False)
    return out


# revision 51
# speedup vs baseline: 1.1857x; 1.0023x over previous
# Multi-head attention (N=4, S=2048, E=512, H=8, D=64) on 8 NeuronCores.
#
# Sharding: core c -> (batch n = c//2, query half qh = c%2). Each core
# computes attention for its 1024 query rows against the valid keys of
# its batch, all 8 heads, and the full output projection for its rows, so
# outputs are disjoint and no collectives are needed.
#
# Host-side preprocessing (exact, done once per call):
#   - Key compaction: the mask is per-batch over keys; masked keys get
#     exp(-1e9/8) == 0.0f in the reference, so they contribute nothing.
#     K/V/mask are gathered to the valid keys and zero-padded to SK_PAD
#     rows (P(overflow) ~ 5.7 sigma per batch; falls back to the full
#     2048-key module if it ever happens).
#   - Weight folding (pure weight algebra):
#       A  = Wq^T @ Wk / 8          scores = (Xq @ A) @ Xk^T
#       M_h = Wv^T @ Wo[:, h]^T     out += (attn @ Xv_h) @ M_h
#       btot = bo + Wo @ tile(bv,8) exact because attention rows sum to 1
#       bk-term cancels in softmax; bq-term via per-k bias (zero here).
#     A is stored block-diagonally [128,128] so one matmul projects a
#     head pair at once.
#
# Device per core:
#   - PE-transpose Q,K chunks (bf16) to [e, s] layout
#   - XqA^T = blockdiag(A)^T-proj of Q^T per head pair
#   - scores^T[k,q] = K_h^T.T @ XqA_h^T  (PSUM f32)
#   - exp on ACT straight from PSUM with per-partition mask bias -> bf16
#   - AV^T = V_aug.T @ exp^T with a ones column in V giving softmax
#     denominators as row 64; reciprocal on ACT (same act table as Exp),
#     broadcast across partitions with a rank-1 f32r matmul, multiply.
#   - out[q,:] = sum_h AVT_h.T @ M_h (+btot): head pairs at partition
#     bases 0/64 make this a single 128-contraction matmul per slot.

import numpy as np
import ml_dtypes

import concourse.bass as bass
import concourse.tile as tile
from concourse import bacc, mybir
from concourse.bass_utils import run_bass_kernel_spmd
from concourse.masks import make_identity

F32 = mybir.dt.float32
BF16 = mybir.dt.bfloat16
I32 = mybir.dt.int32

H = 8
D = 64
E = 512
N_CORES = 8
FULL_N, FULL_S = 4, 2048
SQ = 1024            # per-core query rows
SK_PAD = 1152        # compacted+padded key rows (9 chunks of 128)
MASK_BIAS = -1.25e8  # == -1e9 / sqrt(64), applied pre-softmax

# Engine facts probed on this runtime: Pool partition_broadcast and custom-DVE
# ops produce garbage / fail ISA checks; DVE divide and Pool
# scalar_tensor_tensor are rejected by walrus; DVE fp8 converts and fp8
# DoubleRow matmuls DO work; stream_shuffle semantics: out[j+32k]=in[mask[j]+32k].


def _split_even(n_items, n_steps):
    """Distribute range(n_items) into n_steps contiguous chunks, sizes
    as even as possible (larger first)."""
    out, base = [], 0
    for s in range(n_steps):
        take = (n_items - base + (n_steps - s - 1)) // (n_steps - s)
        out.append(list(range(base, base + take)))
        base += take
    return out


def _emit(tc, t, SQ, SK, has_qbias):
    nc = tc.nc
    NQC = SQ // 128           # query chunks (transpose granularity)
    NKC = SK // 128           # key chunks
    QGS = min(512, SQ)        # q group size for matmul free dim
    NQG = SQ // QGS
    H0 = (NKC + 1) // 2       # k-chunks in half 0 (exp half-tile)
    H1 = NKC - H0
    F32R = mybir.dt.float32r
    sub, mult, add = (mybir.AluOpType.subtract, mybir.AluOpType.mult,
                      mybir.AluOpType.add)

    with (
        tc.tile_pool(name="singles", bufs=1) as singles,
        tc.tile_pool(name="stage", bufs=6) as stage,
        tc.tile_pool(name="expp", bufs=7) as expp,
        tc.tile_pool(name="small", bufs=2) as small,
        tc.tile_pool(name="outp", bufs=2) as outp,
        tc.tile_pool(name="p_sc", bufs=2, space="PSUM") as p_sc,
        tc.tile_pool(name="p_misc", bufs=4, space="PSUM") as p_misc,
    ):
        # ---- constants / weights ----
        ident = singles.tile([128, 128], BF16)
        make_identity(nc, ident)
        identf = singles.tile([128, 128], F32)
        nc.vector.tensor_copy(identf, ident)

        # Dummy matmuls: the PE p-state ramps to max (2.4GHz) only after 3us
        # of CONTINUOUS execution, and any idle gap resets the ramp (the
        # next ~3us then run at 1.2GHz). Burst enough to cover the DMA wait
        # for the first query chunk, and sprinkle more later as gap filler
        # during the load-paced warmup phases.
        warm = p_misc.tile([128, 128], F32, tag="ps")

        def pe_warm_fill(n):
            for _ in range(n):
                nc.tensor.matmul(warm, lhsT=ident, rhs=ident,
                                 start=True, stop=True)

        pe_warm_fill(48)

        # constants ride the gpsimd queue: the two hwdge queues are left
        # free so the first query chunks start transferring immediately
        a_sb = singles.tile([128, 128], BF16)   # blockdiag(A, A)
        nc.gpsimd.dma_start(a_sb, t["a2"][:])
        m_sb = singles.tile([128, 5, E], BF16)   # loaded later, needed by post
        btot_rep = singles.tile([128, E], F32)   # broadcast-loaded later

        # mask -> additive bias, [128, NKC] with k = kt*128 + p
        mask_i = singles.tile([128, NKC], I32)
        nc.gpsimd.dma_start(mask_i, t["mask"][:].rearrange("(kt p) -> p kt", p=128))
        mask_f = singles.tile([128, NKC], F32)
        nc.vector.tensor_copy(mask_f, mask_i)
        mbias = singles.tile([128, NKC], F32)
        # (mask - 1) * (-MASK_BIAS):  mask=0 -> MASK_BIAS, mask=1 -> 0
        nc.vector.tensor_scalar(out=mbias, in0=mask_f, scalar1=1.0,
                                scalar2=-MASK_BIAS, op0=sub, op1=mult)

        # ---- persistent tensors ----
        qt = singles.tile([128, 4, SQ], BF16)    # query^T: e=fc*128+p
        kt = singles.tile([128, 4, SK], BF16)    # key^T
        vt = singles.tile([128, NKC, H, D + 1], BF16)  # value + ones col
        xqa = singles.tile([128, 4, SQ], BF16)   # (Xq @ A)^T per head
        # normalized (attn @ V)^T: head pair layout, odd heads at base 64
        avt = singles.tile([128, 4, SQ], BF16)

        # ---- chunk loader: load f32, cast bf16, PE-transpose into dst ----
        def load_transposed_chunk(src, dst, c, queue, queue2=None):
            raw = stage.tile([128, E], F32, tag="ld")
            if queue2 is not None:
                # split across both hwdge queues: halves transfer in
                # parallel, halving the chunk's arrival latency
                queue.dma_start(raw[:, 0:E // 2],
                                src[c * 128:(c + 1) * 128, 0:E // 2])
                queue2.dma_start(raw[:, E // 2:E],
                                 src[c * 128:(c + 1) * 128, E // 2:E])
            else:
                queue.dma_start(raw, src[c * 128:(c + 1) * 128, :])
            cast = stage.tile([128, E], BF16, tag="cast")
            nc.vector.tensor_copy(cast, raw)
            tp = p_sc.tile([128, 4, 128], BF16, tag="sc")
            for eg in range(4):
                nc.tensor.transpose(tp[:, eg, :],
                                    cast[:, eg * 128:(eg + 1) * 128], ident)
            nc.vector.tensor_copy(dst[:, :, c * 128:(c + 1) * 128], tp)

        # ---- Q: load + transpose, then the A-projection per head pair ----
        # alternate full-chunk loads over both hwdge queues (the transfers
        # are descriptor/latency-bound: halving them into column splits
        # doubles descriptor overhead and is slower)
        def emit_xqa(g):
            gsl = slice(g * QGS, (g + 1) * QGS)
            for fc in range(4):
                ps = p_sc.tile([128, QGS], F32, tag="sc")
                nc.tensor.matmul(ps, lhsT=a_sb, rhs=qt[:, fc, gsl],
                                 start=True, stop=True)
                nc.vector.tensor_copy(xqa[:, fc, gsl], ps)

        # emit each q-group's A-projection as soon as its 4 chunks are
        # loaded: the sc ring assigns slots in emission order, so emitting
        # xqa after all 8 transposes would falsely gate g=0 on chunk 6/7
        qqs = [nc.sync, nc.scalar]
        for c in range(NQC):
            load_transposed_chunk(t["query"][:], qt, c, qqs[c % 2])
            pe_warm_fill(10)
            if c == 3:
                emit_xqa(0)
        emit_xqa(1)

        # ---- optional exact bq correction: per-(h,k) additive bias ----
        # scores^T gains (Xk_h @ (Wk^T bq / 8))[k], constant over q.
        if has_qbias:
            GW = 512 if SK % 512 == 0 else 384
            for c in range(NKC):
                load_transposed_chunk(t["key"][:], kt, c, nc.gpsimd)
            w2 = singles.tile([128, 1], BF16)
            nc.sync.dma_start(w2, t["w2"][:])
            hbias = []
            for h in range(H):
                bp, fc = 64 * (h % 2), h // 2
                row = small.tile([1, SK], F32, tag="hb_row")
                for g in range(SK // GW):
                    ps = p_misc.tile([128, GW], F32, tag="ps")
                    nc.tensor.matmul(ps[0:1, :], lhsT=w2[bp:bp + 64, :],
                                     rhs=kt[bp:bp + 64, fc, g * GW:(g + 1) * GW],
                                     start=True, stop=True)
                    nc.vector.tensor_copy(row[:, g * GW:(g + 1) * GW],
                                          ps[0:1, :])
                hb = singles.tile([128, NKC], F32, name=f"hbias{h}")
                nc.gpsimd.dma_start(hb, row[0, :].rearrange("(kt p) -> p kt",
                                                            p=128))
                nc.vector.tensor_tensor(out=hb, in0=hb, in1=mbias, op=add)
                hbias.append(hb)
        else:
            hbias = [mbias] * H

        # ---- per head-pair: scores^T -> exp -> AV^T, row-group alternated ---
        # K-chunk transposes are folded into pair 0; attn@V matmuls of the
        # previous pair's heads are folded into later pairs for PE smoothness.
        av_state = {}

        def av_alloc(h, pool=None, tag="ps"):
            pool = pool or p_misc
            av_state[h] = [pool.tile([128, QGS], F32, tag=tag,
                                     name=f"av{h}g{g}") for g in range(NQG)]

        def av_mm(h, exA, exB, c):
            ex, ci = (exA, c) if c < H0 else (exB, c - H0)
            for g in range(NQG):
                nc.tensor.matmul(av_state[h][g][0:D + 1, :],
                                 lhsT=vt[:, c, h, :],
                                 rhs=ex[:, ci, g * QGS:(g + 1) * QGS],
                                 start=(c == 0), stop=(c == NKC - 1))

        # Normalization runs in two stages so the expensive DVE reciprocal
        # (a ucode op, ~6.5ns/elem regardless of partition count) is batched
        # over a whole head pair: av_copy(h) drains PSUM per head and DMAs
        # its denominator rows into a gather tile; av_finish(b) reciprocals
        # all 2*NQG rows of the pair in ONE DVE op, broadcasts each across
        # partitions with a rank-1 f32r matmul, and multiplies.
        avsb_t = {}
        den_t = {}

        # den gather tiles for batches 0..2 (stream_shuffle maps
        # out[j+32k] = in[mask[j]+32k], so each (h,g) denominator row is
        # replicated at partitions i and i+32); memsets run upfront on an
        # idle DVE instead of crowding the pair boundaries
        for b in range(3):
            den_t[b] = small.tile([64, QGS], F32, tag="den", bufs=3,
                                  name=f"den{b}")
            nc.vector.memset(den_t[b], 1.0)

        def av_copy(h, gather_den=True):
            b = h // 2
            for g in range(NQG):
                ps = av_state[h][g]
                i = (h % 2) * NQG + g
                avsb = small.tile([128, QGS], F32R, tag="avsb", bufs=4,
                                  name=f"avsb{h}g{g}")
                nc.vector.tensor_copy(avsb[0:D + 1, :], ps[0:D + 1, :])
                if gather_den:
                    # (hwdge queues regress here: the scalar queue is issued
                    # by the ACT sequencer, which is saturated with EXP work)
                    den_row = avsb[D:D + 1, :].bitcast(F32)
                    nc.gpsimd.dma_start(den_t[b][i:i + 1, :], den_row)
                    nc.gpsimd.dma_start(den_t[b][i + 32:i + 33, :], den_row)
                avsb_t[(h, g)] = avsb
            del av_state[h]

        rec_t = {}

        def _norm_head(b, h, rec):
            # rec rows for head h hold the reciprocal'd denominators at
            # partitions i and i+32; stream_shuffle (DVE partition crossbar)
            # broadcasts them to all 64 partitions, then multiply. The whole
            # chain is DVE+DMA only, so the PE stream never stalls on it.
            fc = h // 2
            for g in range(NQG):
                i = (h % 2) * NQG + g
                avsb = avsb_t.pop((h, g))
                bc = small.tile([64, QGS], F32, tag="bc", bufs=4,
                                name=f"bc{b}i{i}")
                nc.vector.stream_shuffle(bc, rec, [i] * 32)
                gsl = slice(g * QGS, (g + 1) * QGS)
                if h % 2 == 0:
                    nc.vector.tensor_tensor(out=avt[0:D, fc, gsl],
                                            in0=avsb[0:D, :].bitcast(F32),
                                            in1=bc, op=mult)
                else:
                    avtmp = small.tile([64, QGS], BF16, tag="avtmp")
                    nc.vector.tensor_tensor(out=avtmp,
                                            in0=avsb[0:D, :].bitcast(F32),
                                            in1=bc, op=mult)
                    nc.gpsimd.dma_start(avt[64:64 + D, fc, gsl], avtmp)

        def av_finish(b):
            # one batched reciprocal for the whole pair, then normalize the
            # odd head first so its extra SBUF->SBUF partition-hop DMA
            # overlaps the even head's direct writes
            rec = small.tile([64, QGS], F32, tag="rec", bufs=2,
                             name=f"rec{b}")
            nc.vector.reciprocal(rec, den_t[b])
            _norm_head(b, 2 * b + 1, rec)
            _norm_head(b, 2 * b, rec)
            del den_t[b]

        def av_finish_part(b, h):
            # per-head half-batch: reciprocal the whole gather tile as soon
            # as this head's rows are in (the other head's rows are still
            # memset 1.0), so the first head's norm chain hides under the
            # second head's AV matmuls on PE
            rec = small.tile([64, QGS], F32, tag="rec", bufs=2,
                             name=f"rec{b}h{h}")
            nc.vector.reciprocal(rec, den_t[b])
            _norm_head(b, h, rec)

        def emit_half(p, half, av_head=None, av_tiles=None, ktrans=False,
                      norm_cb=None):
            h0, h1 = 2 * p, 2 * p + 1
            hsz, base = (H0, 0) if half == 0 else (H1, H0)
            # norm work for earlier heads is emitted BEFORE this half's
            # av_alloc: its pb tiles then reuse p_misc ring slots whose
            # drain copies were emitted in the same callback, keeping the
            # per-engine FIFOs acyclic (emitting it later deadlocks)
            if norm_cb is not None:
                norm_cb()
            # one tag -> one ring of bufs; allocate max-half shape always
            exs = [expp.tile([128, H0, SQ], BF16, tag="exp",
                             name=f"exp_{h}_{half}") for h in (h0, h1)]
            if av_head is not None and av_head not in av_state:
                av_alloc(av_head)
            av_sched = _split_even(NKC, hsz)
            for kt_l in range(hsz):
                kt_i = base + kt_l
                if ktrans:
                    load_transposed_chunk(t["key"][:], kt, kt_i, nc.gpsimd)
                pss = [p_sc.tile([128, SQ], F32, tag="sc", name=f"sc{j}")
                       for j in range(2)]
                # j outer, g inner: consecutive matmuls share lhsT, halving
                # LDWEIGHTS swaps, and pss[0] completes before pss[1] starts
                # so its exp overlaps the second head's matmuls
                for j, h in enumerate((h0, h1)):
                    bp, fc = 64 * (h % 2), h // 2
                    for g in range(NQG):
                        nc.tensor.matmul(
                            pss[j][:, g * QGS:(g + 1) * QGS],
                            lhsT=kt[bp:bp + 64, fc,
                                    kt_i * 128:(kt_i + 1) * 128],
                            rhs=xqa[bp:bp + 64, fc, g * QGS:(g + 1) * QGS],
                            start=True, stop=True)
                if av_head is not None:
                    for c in av_sched[kt_l]:
                        av_mm(av_head, av_tiles[0], av_tiles[1], c)
                for j, h in enumerate((h0, h1)):
                    nc.scalar.activation(out=exs[j][:, kt_l, :], in_=pss[j],
                                         func=mybir.ActivationFunctionType.Exp,
                                         bias=hbias[h][:, kt_i:kt_i + 1],
                                         scale=1.0)
            return exs

        halves = {}

        def set_halves(p, half, exs):
            halves[(2 * p, half)], halves[(2 * p + 1, half)] = exs

        def pop_head(h):
            return halves.pop((h, 0)), halves.pop((h, 1))

        set_halves(0, 0, emit_half(0, 0, ktrans=not has_qbias))
        pe_warm_fill(24)
        set_halves(0, 1, emit_half(0, 1, ktrans=not has_qbias))
        pe_warm_fill(16)
        # V: load + cast while pair-1 scores run (needed first by AV(h0))
        nc.gpsimd.dma_start(btot_rep,
                            t["btot"][:][None, :].to_broadcast([128, E]))
        nc.vector.memset(vt[:, :, :, D:D + 1], 1.0)
        for c in range(NKC):
            raw = stage.tile([128, E], F32, tag="ld")
            nc.sync.dma_start(raw, t["value"][c * 128:(c + 1) * 128, :])
            nc.vector.tensor_copy(vt[:, c, :, 0:D],
                                  raw.rearrange("p (h d) -> p h d", h=H))
        norm_cbs = {
            (1, 1): lambda: av_copy(0),
            (2, 0): lambda: (av_copy(1), av_finish(0)),
            (2, 1): lambda: av_copy(2),
            (3, 0): lambda: (av_copy(3), av_finish(1)),
            (3, 1): lambda: av_copy(4),
        }
        for p in range(1, H // 2):
            hh = 2 * (p - 1)
            ta, tb = pop_head(hh)
            set_halves(p, 0, emit_half(p, 0, av_head=hh, av_tiles=(ta, tb),
                                       norm_cb=norm_cbs.get((p, 0))))
            ta, tb = pop_head(hh + 1)
            set_halves(p, 1, emit_half(p, 1, av_head=hh + 1,
                                       av_tiles=(ta, tb),
                                       norm_cb=norm_cbs.get((p, 1))))
        nc.sync.dma_start(m_sb, t["m2"][:])
        t6, t7 = pop_head(H - 2), pop_head(H - 1)
        av_copy(5)
        av_finish(2)

        # ---- output projection pass 1: head slots 0..2 are done; fold them
        # (plus the output bias, so pass 2 needs only one add) into a f32
        # accumulator while the last exps drain on ACT. Head pairs sit at
        # partition bases 0/64 of avt and m_sb, so a single 128-contraction
        # matmul sums both heads of a slot.
        out_acc = singles.tile([128, NQC, E], F32)
        for q_i in range(SQ // 128):
            qs = slice(q_i * 128, (q_i + 1) * 128)
            psA = p_misc.tile([128, E], F32, tag="ps")
            for j in range(3):
                nc.tensor.matmul(psA, lhsT=avt[:, j, qs], rhs=m_sb[:, j, :],
                                 start=(j == 0), stop=(j == 2))
            nc.vector.tensor_tensor(out=out_acc[:, q_i, :], in0=psA,
                                    in1=btot_rep, op=add)
        # ---- last pair: normalize AFTER the output projection ----
        # The out-proj result has q on partitions, so 1/den becomes a
        # per-PARTITION scale: transpose the denominator rows into columns
        # on PE (16 tiny transposes), reciprocal them all in ONE cheap DVE
        # op ([128,16], free-size-bound: ~0.1us vs 2x3.35us), and fold the
        # scale + accumulate into fused scalar_tensor_tensor ops. No
        # serial reciprocal, no shuffles, no partition-hop DMA in the tail.
        av_alloc(H - 2)
        for c in range(NKC):
            av_mm(H - 2, t6[0], t6[1], c)
        av_copy(H - 2, gather_den=False)
        av_alloc(H - 1)
        for c in range(NKC):
            av_mm(H - 1, t7[0], t7[1], c)
        av_copy(H - 1, gather_den=False)

        # unnormalized bf16 AV for the last pair, each head at base 0
        avu = {}
        for h in (H - 2, H - 1):
            for g in range(NQG):
                avsb = avsb_t.pop((h, g))
                u = small.tile([64, QGS], BF16, tag="avu", bufs=4,
                               name=f"avu{h}g{g}")
                nc.vector.tensor_copy(u, avsb[0:D, :].bitcast(F32))
                avu[(h, g)] = (u, avsb)

        # denominator rows -> columns: rank-1 matmuls (out[i,0] =
        # row[0,i] * 1.0; the 1x1 identity slice sits at base partition 64
        # to match the row's base)
        dcol = p_sc.tile([128, SQ], F32, tag="sc")
        for hi, h in enumerate((H - 2, H - 1)):
            for g in range(NQG):
                avsb = avu[(h, g)][1]
                for c in range(4):
                    idx = hi * 8 + g * 4 + c
                    nc.tensor.matmul(
                        dcol[:, idx:idx + 1],
                        lhsT=avsb[D:D + 1, c * 128:(c + 1) * 128].bitcast(F32),
                        rhs=identf[64:65, 64:65],
                        start=True, stop=True)
        rcol = small.tile([128, 16], F32)
        nc.vector.reciprocal(rcol, dcol[:, 0:16])

        # per-head out-proj + fused scale/accumulate, then store
        for q_i in range(SQ // 128):
            qs = slice(q_i * 128, (q_i + 1) * 128)
            g, c = q_i // 4, q_i % 4
            csl = slice(c * 128, (c + 1) * 128)
            ps6 = p_misc.tile([128, E], F32, tag="ps")
            nc.tensor.matmul(ps6, lhsT=avu[(H - 2, g)][0][:, csl],
                             rhs=m_sb[0:D, 3, :], start=True, stop=True)
            ps7 = p_misc.tile([128, E], F32, tag="ps")
            nc.tensor.matmul(ps7, lhsT=avu[(H - 1, g)][0][:, csl],
                             rhs=m_sb[0:D, 4, :], start=True, stop=True)
            ob = outp.tile([128, E], F32, tag="ob", bufs=4)
            nc.vector.scalar_tensor_tensor(
                out=ob, in0=ps6, scalar=rcol[:, q_i:q_i + 1],
                in1=out_acc[:, q_i, :], op0=mult, op1=add)
            ob2 = outp.tile([128, E], F32, tag="ob2", bufs=4)
            nc.vector.scalar_tensor_tensor(
                out=ob2, in0=ps7, scalar=rcol[:, 8 + q_i:9 + q_i],
                in1=ob, op0=mult, op1=add)
            nc.sync.dma_start(t["out"][qs, :], ob2)


def build_module(SQ=SQ, SK=SK_PAD, has_qbias=False):
    nc = bacc.Bacc()
    t = {
        "query": nc.dram_tensor("query", [SQ, E], F32, kind="ExternalInput"),
        "key": nc.dram_tensor("key", [SK, E], F32, kind="ExternalInput"),
        "value": nc.dram_tensor("value", [SK, E], F32, kind="ExternalInput"),
        "mask": nc.dram_tensor("mask", [SK], I32, kind="ExternalInput"),
        "a2": nc.dram_tensor("a2", [128, 128], BF16, kind="ExternalInput"),
        "m2": nc.dram_tensor("m2", [128, 5, E], BF16, kind="ExternalInput"),
        "btot": nc.dram_tensor("btot", [E], F32, kind="ExternalInput"),
        "out": nc.dram_tensor("out", [SQ, E], F32, kind="ExternalOutput"),
    }
    if has_qbias:
        t["w2"] = nc.dram_tensor("w2", [128, 1], BF16, kind="ExternalInput")
    with tile.TileContext(nc) as tc:
        _emit(tc, t, SQ, SK, has_qbias)
    nc.compile()
    return nc


_MODULE_CACHE = {}


def _get_module(SQ, SK, has_qbias):
    key = (SQ, SK, has_qbias)
    if key not in _MODULE_CACHE:
        _MODULE_CACHE[key] = build_module(SQ, SK, has_qbias)
    return _MODULE_CACHE[key]


def _fold_weights(Wq, Wk, Wv, Wo, bv, bo):
    Wq, Wk, Wv, Wo = (np.asarray(w, np.float64) for w in (Wq, Wk, Wv, Wo))
    A = (Wq.T @ Wk) / np.sqrt(np.float64(D))
    a2 = np.zeros((128, 128), np.float64)   # blockdiag: one matmul per pair
    a2[0:D, 0:D] = A
    a2[D:2 * D, D:2 * D] = A
    a2 = a2.astype(ml_dtypes.bfloat16)
    Ms = [Wv.T @ Wo[:, h * D:(h + 1) * D].T for h in range(H)]
    # head-pair packing: head h at partitions 64*(h%2) .. +64, free slot
    # h//2; slot 4 repeats M7 at base 0 for the per-head tail projection
    m2 = np.zeros((128, 5, E), np.float64)
    for h in range(H):
        m2[64 * (h % 2):64 * (h % 2) + D, h // 2, :] = Ms[h]
    m2[0:D, 4, :] = Ms[H - 1]
    m2 = m2.astype(ml_dtypes.bfloat16)
    btot = (np.asarray(bo, np.float64)
            + Wo @ np.tile(np.asarray(bv, np.float64), H)).astype(np.float32)
    return a2, m2, btot


def _run(inputs, trace=False):
    query = np.asarray(inputs["query"], np.float32)
    key = np.asarray(inputs["key"], np.float32)
    value = np.asarray(inputs["value"], np.float32)
    mask = np.asarray(inputs["mask"])
    a2, m2, btot = _fold_weights(inputs["Wq"], inputs["Wk"], inputs["Wv"],
                                 inputs["Wo"], inputs["bv"], inputs["bo"])
    bq = np.asarray(inputs["bq"], np.float64)
    bk = np.asarray(inputs["bk"], np.float64)  # noqa: F841  (cancels in softmax)
    has_qbias = bool(np.any(bq != 0))
    w2 = None
    if has_qbias:
        w2v = (np.asarray(inputs["Wk"], np.float64).T @ bq) / np.sqrt(float(D))
        w2 = np.concatenate([w2v, w2v]).reshape(128, 1).astype(ml_dtypes.bfloat16)

    n_batch, S = query.shape[0], query.shape[1]
    sq = S // 2

    # Per-batch key compaction: gather valid keys, zero-pad to SK_PAD.
    # exp(MASK_BIAS + s) == 0.0f for the pad rows, so this is exact.
    mask2d = mask.reshape(n_batch, S).astype(np.int32)
    n_valid = [int(np.count_nonzero(mask2d[n])) for n in range(n_batch)]
    if max(n_valid) <= SK_PAD and S >= SK_PAD:
        sk = SK_PAD
        key_c = np.zeros((n_batch, sk, E), np.float32)
        val_c = np.zeros((n_batch, sk, E), np.float32)
        mask_c = np.zeros((n_batch, sk), np.int32)
        for n in range(n_batch):
            idx = np.nonzero(mask2d[n])[0]
            key_c[n, :len(idx)] = key[n][idx]
            val_c[n, :len(idx)] = value[n][idx]
            mask_c[n, :len(idx)] = 1
    else:
        sk = S
        key_c, val_c, mask_c = key, value, mask2d

    nc = _get_module(sq, sk, has_qbias)

    in_maps = []
    for c in range(N_CORES):
        n, qh = divmod(c, 2)
        m = {
            "query": np.ascontiguousarray(query[n, qh * sq:(qh + 1) * sq, :]),
            "key": np.ascontiguousarray(key_c[n]),
            "value": np.ascontiguousarray(val_c[n]),
            "mask": np.ascontiguousarray(mask_c[n]),
            "a2": a2, "m2": m2, "btot": btot,
        }
        if has_qbias:
            m["w2"] = w2
        in_maps.append(m)

    res = run_bass_kernel_spmd(nc, in_maps, core_ids=list(range(N_CORES)),
                               trace=trace)
    out = np.empty((n_batch, S, E), np.float32)
    for c, r in enumerate(res.results):
        n, qh = divmod(c, 2)
        out[n, qh * sq:(qh + 1) * sq, :] = r["out"]
    return out, res


def kernel(**inputs) -> np.ndarray:
    out, _ = _run(inputs, trace=False)
    return out
